# revision 1
# baseline (speedup 1.0000x reference)
"""LocalVariation kernel for Trainium2 (8 NeuronCores, data-parallel over batch).

out[b, k, y, x] = x[b, 0, y, x] - xp[b, 0, y + di, x + dj]   (replicate pad)
for the 24 off-center (di, dj) offsets of a 5x5 window.

Sharding: batch 16 -> 2 images per core. The host pre-pads each image to
[516, 516] (replicate). The device program is built to minimize instruction
count (the execution environment is dominated by fixed per-instruction cost):

  - ONE 5.2-MiB DMA per image loads T[p, c, i, x] = xpad[128c + p + i, x]
    (the overlapping-window source AP merges (i, x) into one contiguous dim).
  - ONE DVE tensor_sub per 128-row chunk computes all 25 (i, j) blocks at
    once via a 3-free-dim window access pattern (the center block is zeros
    and is simply not stored).
  - TWO 3-MiB stores per chunk (12 channels each), alternating between the
    sync and scalar HWDGE rings.
"""

import numpy as np

import concourse.bass as bass
import concourse.bacc as bacc
import concourse.mybir as mybir
import concourse.tile as tile
from concourse.bass_utils import run_bass_kernel_spmd

N_CORES = 8
B_FULL = 16
BPC = B_FULL // N_CORES  # images per core
H = W = 512
KSZ = 5
PAD = 2
NBR = KSZ * KSZ - 1  # 24
HP = H + 2 * PAD  # 516
WP = W + 2 * PAD  # 516
F32 = mybir.dt.float32
NCH = H // 128  # 4 chunks per image
CBLK = KSZ * WP  # free elems per (chunk) block in T: 2580

_NC_CACHE = {}


def _build_image(nc, tin, tout, x, out, b):
    # One load for the whole image: T[p, c, i*WP + x] = xpad[b, 128c + p + i, x]
    T = tin.tile([128, NCH, CBLK], F32, name=f"T_{b}", tag="T")
    pstep = T.ap[0][0]
    nc.gpsimd.dma_start(
        out=T[:, :, :],
        in_=bass.AP(
            x, b * HP * WP, [[WP, 128], [128 * WP, NCH], [WP, KSZ], [1, WP]]
        ),
    )

    for c in range(NCH):
        # O[p, 5i+j, x] = center - T[p, c, i, j + x]  (one DVE op, FD=12800)
        O = tout.tile([128, KSZ * KSZ, W], F32, name=f"O_{b}_{c}", tag="O")
        ostep = O.ap[0][0]
        tbase = T.offset + c * CBLK
        center = bass.AP(
            T.tensor, tbase + PAD * WP + PAD, [[pstep, 128], [0, KSZ], [0, KSZ], [1, W]]
        )
        win = bass.AP(T.tensor, tbase, [[pstep, 128], [WP, KSZ], [1, KSZ], [1, W]])
        o3 = bass.AP(O.tensor, O.offset, [[ostep, 128], [KSZ * W, KSZ], [W, KSZ], [1, W]])
        nc.vector.tensor_sub(o3, center, win)

        # two stores (channels 0..11 from blocks 0..11, 12..23 from 13..24)
        ooff = b * NBR * H * W + 128 * c * W
        eng1, eng2 = (nc.sync, nc.scalar) if c % 2 == 0 else (nc.scalar, nc.sync)
        eng1.dma_start(
            out=bass.AP(out, ooff, [[W, 128], [H * W, 12], [1, W]]),
            in_=O[:, 0:12, :],
        )
        eng2.dma_start(
            out=bass.AP(out, ooff + 12 * H * W, [[W, 128], [H * W, 12], [1, W]]),
            in_=O[:, 13:25, :],
        )


def build(reps=1, tiny_out=False):
    """tiny_out=True: bench variant — full-size stores go to an Internal DRAM
    tensor (same HBM traffic) and only a [128, 512] probe is an ExternalOutput,
    so per-call transfer over the axon tunnel is negligible."""
    nc = bacc.Bacc("TRN2", target_bir_lowering=False, debug=False, num_devices=N_CORES)
    x = nc.dram_tensor("x", [BPC, HP, WP], F32, kind="ExternalInput")
    out_kind = "Internal" if tiny_out else "ExternalOutput"
    out = nc.dram_tensor("out", [BPC, NBR, H, W], F32, kind=out_kind)
    probe = (
        nc.dram_tensor("probe", [128, W], F32, kind="ExternalOutput") if tiny_out else None
    )
    with tile.TileContext(nc) as tc:
        with (
            tc.tile_pool(name="tin", bufs=2) as tin,
            tc.tile_pool(name="tout", bufs=2) as tout,
        ):
            for _ in range(reps):
                for b in range(BPC):
                    _build_image(nc, tin, tout, x, out, b)
            if probe is not None:
                pt = tin.tile([128, W], F32, name="pt", tag="pt")
                nc.sync.dma_start(out=pt[:, :], in_=bass.AP(out, 0, [[W, 128], [1, W]]))
                nc.sync.dma_start(out=probe.ap(), in_=pt[:, :])
    nc.compile()
    return nc


def _get_nc():
    if "nc" not in _NC_CACHE:
        _NC_CACHE["nc"] = build()
    return _NC_CACHE["nc"]


def pad_input(x):
    """[16, 1, 512, 512] -> replicate-padded [16, 516, 516], float32."""
    xs = np.asarray(x, dtype=np.float32).reshape(B_FULL, H, W)
    return np.pad(xs, ((0, 0), (PAD, PAD), (PAD, PAD)), mode="edge")


def run(x, trace=False):
    nc = _get_nc()
    xp = pad_input(x)
    in_maps = [
        {"x": np.ascontiguousarray(xp[BPC * i : BPC * (i + 1)])} for i in range(N_CORES)
    ]
    res = run_bass_kernel_spmd(nc, in_maps, core_ids=list(range(N_CORES)), trace=trace)
    full = np.concatenate([res.results[i]["out"] for i in range(N_CORES)], axis=0)
    return full.reshape(B_FULL, NBR, H, W), res


def kernel(x):
    return run(x)[0]



# revision 2
# speedup vs baseline: 1.0404x; 1.0404x over previous
"""LocalVariation kernel for Trainium2 (8 NeuronCores, data-parallel over batch).

out[b, k, y, x] = x[b, 0, y, x] - xp[b, 0, y + di, x + dj]   (replicate pad)
for the 24 off-center (di, dj) offsets of a 5x5 window.

Sharding: batch 16 -> 2 images per core. The host replicate-pads each image
to [516, 516] and casts to bf16 (the 2e-2 rel-err budget leaves bf16 with an
~8x margin; measured rel err 2.5e-3). The kernel is pure data movement plus
one subtract per output element, so it is output-bandwidth-bound; halving
the store bytes with bf16 halves the runtime.

Layout: partition p holds output rows 4p..4p+3 (RPP=4). Each store DMA
writes one 6-channel group; the HBM-side contiguous run per (partition,
channel) is 4 rows = 4KB, and one channel image (512 rows) spans exactly
128 partitions, so the whole group is a single 3-dim AP. All stores ride
the sync HWDGE ring (measured faster than alternating sync/scalar); loads
ride the scalar ring. DVE tensor_subs use all 128 partitions with free
dims (j-run, row, x), one op per kernel-row segment of a channel group.

Per image: 1 load (1.1MB) + 8 DVE subs + 4 stores (3.1MB each).
Measured: ~77us/rep per core at 8 cores concurrent (~356 GB/s/core), vs
the HBM roofline of ~71us for the store bytes alone.
"""

import ml_dtypes
import numpy as np

import concourse.bass as bass
import concourse.bacc as bacc
import concourse.mybir as mybir
import concourse.tile as tile
from concourse.bass_utils import run_bass_kernel_spmd

N_CORES = 8
B_FULL = 16
BPC = B_FULL // N_CORES  # images per core
H = W = 512
KSZ = 5
PAD = 2
NBR = KSZ * KSZ - 1  # 24
HP = H + 2 * PAD  # 516
WP = W + 2 * PAD  # 516
F32 = mybir.dt.float32
BF16 = mybir.dt.bfloat16
RPP = 4  # output rows per partition
HALO = RPP + 2 * PAD  # 8 rows of padded input per partition
IMG = H * W  # elems per output channel image (262144)

_NC_CACHE = {}


def _kgroup_ops(g):
    """DVE op splits for channel group g (channels 6g..6g+5).

    Returns list of (i, j0, nj, c0): kernel-row i, first col j0, run length
    nj, position c0 of the run within the group. Runs never cross kernel-row
    boundaries (the j free-dim stride is 1 element)."""
    ops = []
    ks = list(range(6 * g, 6 * g + 6))
    idxs = [k + (1 if k >= 12 else 0) for k in ks]  # skip center (2,2)=12
    c0 = 0
    while c0 < 6:
        idx = idxs[c0]
        i, j0 = idx // KSZ, idx % KSZ
        nj = 1
        while c0 + nj < 6 and idxs[c0 + nj] == idx + nj and (idx + nj) // KSZ == i:
            nj += 1
        ops.append((i, j0, nj, c0))
        c0 += nj
    return ops


def _build_image(nc, tin, tout, x, out, b):
    # xin[p, r, xx] = xpad[b, 4p + r, xx], r in 0..8 (rows 4p..4p+7)
    xin = tin.tile([128, HALO, WP], BF16, name=f"xin_{b}", tag="xin")
    xstep = xin.ap[0][0]
    nc.scalar.dma_start(
        out=xin[:, :, :],
        in_=bass.AP(x, b * HP * WP, [[RPP * WP, 128], [1, HALO * WP]]),
    )

    for g in range(4):
        S = tout.tile([128, 6, RPP, W], BF16, name=f"S_{b}_{g}", tag="S")
        sstep = S.ap[0][0]
        for i, j0, nj, c0 in _kgroup_ops(g):
            oap = bass.AP(
                S.tensor,
                S.offset + c0 * RPP * W,
                [[sstep, 128], [RPP * W, nj], [W, RPP], [1, W]],
            )
            center = bass.AP(
                xin.tensor,
                xin.offset + PAD * WP + PAD,
                [[xstep, 128], [0, nj], [WP, RPP], [1, W]],
            )
            win = bass.AP(
                xin.tensor,
                xin.offset + i * WP + j0,
                [[xstep, 128], [1, nj], [WP, RPP], [1, W]],
            )
            nc.vector.tensor_sub(oap, center, win)

        nc.sync.dma_start(
            out=bass.AP(
                out,
                (b * NBR + 6 * g) * IMG,
                [[RPP * W, 128], [IMG, 6], [1, RPP * W]],
            ),
            in_=S[:, :, :, :],
        )


def build(reps=1, tiny_out=False, hw_loop=False, unroll=2):
    """tiny_out=True: bench variant — full-size stores go to an Internal DRAM
    tensor (same HBM traffic) and only a [128, 512] probe is an ExternalOutput,
    so per-call transfer over the axon tunnel is negligible.

    hw_loop=True: wrap the body in a tc.For_i hardware loop executing `reps`
    iterations of `unroll` kernel bodies each — lets the bench run thousands
    of reps with a small NEFF so device time dominates call overhead."""
    nc = bacc.Bacc("TRN2", target_bir_lowering=False, debug=False, num_devices=N_CORES)
    x = nc.dram_tensor("x", [BPC, HP, WP], BF16, kind="ExternalInput")
    out_kind = "Internal" if tiny_out else "ExternalOutput"
    out = nc.dram_tensor("out", [BPC, NBR, H, W], BF16, kind=out_kind)
    probe = (
        nc.dram_tensor("probe", [128, W], BF16, kind="ExternalOutput")
        if tiny_out
        else None
    )
    with tile.TileContext(nc) as tc:
        with (
            tc.tile_pool(name="tin", bufs=3) as tin,
            tc.tile_pool(name="tout", bufs=3) as tout,
        ):
            if hw_loop:
                with tc.For_i(0, reps) as _i:
                    for _ in range(unroll):
                        for b in range(BPC):
                            _build_image(nc, tin, tout, x, out, b)
            else:
                for _ in range(reps):
                    for b in range(BPC):
                        _build_image(nc, tin, tout, x, out, b)
            if probe is not None:
                pt = tin.tile([128, W], BF16, name="pt", tag="pt")
                nc.sync.dma_start(out=pt[:, :], in_=bass.AP(out, 0, [[W, 128], [1, W]]))
                nc.sync.dma_start(out=probe.ap(), in_=pt[:, :])
    nc.compile()
    return nc


def _get_nc():
    if "nc" not in _NC_CACHE:
        _NC_CACHE["nc"] = build()
    return _NC_CACHE["nc"]


def pad_input(x):
    """[16, 1, 512, 512] -> replicate-padded bf16 [16, 516, 516]."""
    xs = np.asarray(x, dtype=np.float32).reshape(B_FULL, H, W)
    xp = np.pad(xs, ((0, 0), (PAD, PAD), (PAD, PAD)), mode="edge")
    return xp.astype(ml_dtypes.bfloat16)


def run(x, trace=False):
    nc = _get_nc()
    xp = pad_input(x)
    in_maps = [
        {"x": np.ascontiguousarray(xp[BPC * i : BPC * (i + 1)])} for i in range(N_CORES)
    ]
    res = run_bass_kernel_spmd(nc, in_maps, core_ids=list(range(N_CORES)), trace=trace)
    full = np.concatenate([res.results[i]["out"] for i in range(N_CORES)], axis=0)
    return full.reshape(B_FULL, NBR, H, W).astype(np.float32), res


def kernel(x):
    return run(x)[0]


# revision 5
# speedup vs baseline: 1.1388x; 1.0946x over previous
"""LocalVariation kernel for Trainium2 (8 NeuronCores, data-parallel over batch).

out[b, k, y, x] = x[b, 0, y, x] - xp[b, 0, y + di, x + dj]   (replicate pad)
for the 24 off-center (di, dj) offsets of a 5x5 window.

Sharding: batch 16 -> 2 images per core. The host replicate-pads each image
to [516, 516] and casts to bf16 (the 2e-2 rel-err budget leaves bf16 with an
~8x margin; measured rel err 2.5e-3). The kernel is pure data movement plus
one subtract per output element, so it is output-bandwidth-bound; halving
the store bytes with bf16 halves the runtime.

Layout: partition p holds output rows 4p..4p+3 (RPP=4). Each store DMA
writes one 6-channel group; the HBM-side contiguous run per (partition,
channel) is 4 rows = 4KB, and one channel image (512 rows) spans exactly
128 partitions, so the whole group is a single 3-dim AP. All stores ride
the sync HWDGE ring (measured faster than alternating sync/scalar); loads
ride the scalar ring. DVE tensor_subs use all 128 partitions with free
dims (j-run, row, x), one op per kernel-row segment of a channel group.

Per image: 1 load (1.1MB) + 8 DVE subs + 8 half-group stores (1.6MB each).
Measured: ~78us/rep per core at 8 cores concurrent (~350 GB/s/core), vs
74.5us for the stores alone (differential attribution; the residue is the
loads' HBM share). GpSimd tensor_sub offload was tried and is slower —
POOL shares its SBUF port with DVE.
"""

import ml_dtypes
import numpy as np

import concourse.bass as bass
import concourse.bacc as bacc
import concourse.mybir as mybir
import concourse.tile as tile
from concourse.bass_utils import run_bass_kernel_spmd

N_CORES = 8
B_FULL = 16
BPC = B_FULL // N_CORES  # images per core
H = W = 512
KSZ = 5
PAD = 2
NBR = KSZ * KSZ - 1  # 24
HP = H + 2 * PAD  # 516
WP = W + 2 * PAD  # 516
F32 = mybir.dt.float32
BF16 = mybir.dt.bfloat16
RPP = 4  # output rows per partition
HALO = RPP + 2 * PAD  # 8 rows of padded input per partition
IMG = H * W  # elems per output channel image (262144)

_NC_CACHE = {}


def _kgroup_ops(g):
    """DVE op splits for channel group g (channels 6g..6g+5).

    Returns list of (i, j0, nj, c0): kernel-row i, first col j0, run length
    nj, position c0 of the run within the group. Runs never cross kernel-row
    boundaries (the j free-dim stride is 1 element)."""
    ops = []
    ks = list(range(6 * g, 6 * g + 6))
    idxs = [k + (1 if k >= 12 else 0) for k in ks]  # skip center (2,2)=12
    c0 = 0
    while c0 < 6:
        idx = idxs[c0]
        i, j0 = idx // KSZ, idx % KSZ
        nj = 1
        while c0 + nj < 6 and idxs[c0 + nj] == idx + nj and (idx + nj) // KSZ == i:
            nj += 1
        ops.append((i, j0, nj, c0))
        c0 += nj
    return ops


def _build_image(nc, tin, tout, x, out, b):
    # xin[p, r, xx] = xpad[b, 4p + r, xx], r in 0..8 (rows 4p..4p+7)
    xin = tin.tile([128, HALO, WP], BF16, name=f"xin_{b}", tag="xin")
    xstep = xin.ap[0][0]
    nc.scalar.dma_start(
        out=xin[:, :, :],
        in_=bass.AP(x, b * HP * WP, [[RPP * WP, 128], [1, HALO * WP]]),
    )

    for g in range(4):
        S = tout.tile([128, 6, RPP, W], BF16, name=f"S_{b}_{g}", tag="S")
        sstep = S.ap[0][0]
        for i, j0, nj, c0 in _kgroup_ops(g):
            oap = bass.AP(
                S.tensor,
                S.offset + c0 * RPP * W,
                [[sstep, 128], [RPP * W, nj], [W, RPP], [1, W]],
            )
            center = bass.AP(
                xin.tensor,
                xin.offset + PAD * WP + PAD,
                [[xstep, 128], [0, nj], [WP, RPP], [1, W]],
            )
            win = bass.AP(
                xin.tensor,
                xin.offset + i * WP + j0,
                [[xstep, 128], [1, nj], [WP, RPP], [1, W]],
            )
            nc.vector.tensor_sub(oap, center, win)

        # two 3-channel stores: the first can start as soon as the first DVE
        # op of the group lands, tightening DVE->store overlap
        for h in range(2):
            nc.sync.dma_start(
                out=bass.AP(
                    out,
                    (b * NBR + 6 * g + 3 * h) * IMG,
                    [[RPP * W, 128], [IMG, 3], [1, RPP * W]],
                ),
                in_=S[:, 3 * h : 3 * h + 3, :, :],
            )


def build(reps=1, tiny_out=False, hw_loop=False, unroll=2):
    """tiny_out=True: bench variant — full-size stores go to an Internal DRAM
    tensor (same HBM traffic) and only a [128, 512] probe is an ExternalOutput,
    so per-call transfer over the axon tunnel is negligible.

    hw_loop=True: wrap the body in a tc.For_i hardware loop executing `reps`
    iterations of `unroll` kernel bodies each — lets the bench run thousands
    of reps with a small NEFF so device time dominates call overhead."""
    nc = bacc.Bacc("TRN2", target_bir_lowering=False, debug=False, num_devices=N_CORES)
    x = nc.dram_tensor("x", [BPC, HP, WP], BF16, kind="ExternalInput")
    out_kind = "Internal" if tiny_out else "ExternalOutput"
    out = nc.dram_tensor("out", [BPC, NBR, H, W], BF16, kind=out_kind)
    probe = (
        nc.dram_tensor("probe", [128, W], BF16, kind="ExternalOutput")
        if tiny_out
        else None
    )
    with tile.TileContext(nc) as tc:
        with (
            tc.tile_pool(name="tin", bufs=3) as tin,
            tc.tile_pool(name="tout", bufs=4) as tout,
        ):
            if hw_loop:
                with tc.For_i(0, reps) as _i:
                    for _ in range(unroll):
                        for b in range(BPC):
                            _build_image(nc, tin, tout, x, out, b)
            else:
                for _ in range(reps):
                    for b in range(BPC):
                        _build_image(nc, tin, tout, x, out, b)
            if probe is not None:
                pt = tin.tile([128, W], BF16, name="pt", tag="pt")
                nc.sync.dma_start(out=pt[:, :], in_=bass.AP(out, 0, [[W, 128], [1, W]]))
                nc.sync.dma_start(out=probe.ap(), in_=pt[:, :])
    nc.compile()
    return nc


def _get_nc():
    if "nc" not in _NC_CACHE:
        _NC_CACHE["nc"] = build()
    return _NC_CACHE["nc"]


def pad_input(x):
    """[16, 1, 512, 512] -> replicate-padded bf16 [16, 516, 516]."""
    xs = np.asarray(x, dtype=np.float32).reshape(B_FULL, H, W)
    xp = np.pad(xs, ((0, 0), (PAD, PAD), (PAD, PAD)), mode="edge")
    return xp.astype(ml_dtypes.bfloat16)


def run(x, trace=False):
    nc = _get_nc()
    xp = pad_input(x)
    in_maps = [
        {"x": np.ascontiguousarray(xp[BPC * i : BPC * (i + 1)])} for i in range(N_CORES)
    ]
    res = run_bass_kernel_spmd(nc, in_maps, core_ids=list(range(N_CORES)), trace=trace)
    full = np.concatenate([res.results[i]["out"] for i in range(N_CORES)], axis=0)
    return full.reshape(B_FULL, NBR, H, W).astype(np.float32), res


def kernel(x):
    return run(x)[0]
